# revision 13
# baseline (speedup 1.0000x reference)
"""Causal self-attention (GPT-style) Bass/Tile kernel for 8 Trainium2 NeuronCores.

Reference computation (fp32):
    qkv = x @ W_attn + b_attn ; q,k,v = split(qkv)
    heads: [B=4, H=16, S=2048, D=64]
    att = softmax(causal(q k^T / sqrt(64)))
    y   = att @ v  -> [B, S, 1024]
    out = y @ W_proj + b_proj

Sharding (hardcoded): 8 cores = 4 batches x 2 head-groups (tensor parallel over
heads).  Core c handles batch c//2, heads 8*(c%2) .. 8*(c%2)+7.  Each core
computes a partial projection output [2048, 1024]; the host sums the two
head-group partials per batch and adds b_proj.

Per-core kernel layout notes:
  - All matmuls run through the PE array as out = lhsT.T @ rhs.  The cost of a
    matmul is proportional to the *output free size* (rhs columns streamed), so
    the kernel trims every attention matmul to the causally-live column range.
  - QKV phase computes q^T / k^T ([feature, seq], feature on partitions) and
    v in [seq, feature] layout, so attention needs no on-chip transposes:
      S^T[j, i] = sum_d kT[d, j] qT[d, i]    (two heads -> two halves of one
                                              2-bank PSUM tile [128, 1024])
      E = exp(S^T / 8) in ONE activation per j-tile (strided AP over both
      heads' live columns); causal triangle masked post-exp (fill 0).
      yT[d, i] (+ row 64 = softmax denom) = [v | 1]^T E  (M=65, K=j)
    Softmax needs no max-subtraction: |S/8| <= ~6 for these inputs.
  - The PE stream is software-pipelined: S(jt+2) issues before PV(jt) so the
    exp of jt overlaps PV(jt-1)/S(jt+1)/filler work instead of stalling PE.
  - Independent "filler" units (next chunk's QKV, or output-projection tiles)
    are paced evenly through the attention j-loops; all projection tiles for
    chunks 0-2 run inside chunk 3 (where the exp deficit is largest).
  - Normalization: denom row -> reciprocal_approx_fast (DVE) ->
    partition_broadcast (GpSimd) -> y * r straight out of PSUM (DVE).
  - bf16 everywhere on the PE; W_proj is cast to bf16 on the host.
"""

import ml_dtypes
import numpy as np

import concourse.bass as bass
import concourse.mybir as mybir
import concourse.tile as tile
from concourse.bass_utils import run_bass_kernel_spmd

F32 = mybir.dt.float32
BF16 = mybir.dt.bfloat16

SL = 2048          # sequence length
ED = 1024          # embed dim
NHC = 8            # heads per core
DH = 64            # head dim
PT = 128           # partitions
CH = 512           # free-dim chunk (PSUM bank)
NCI = SL // CH     # 4 i-chunks
NST = SL // PT     # 16 seq tiles
NKT = ED // PT     # 8 contraction tiles for QKV


def build_kernel(ctx, nc: bass.Bass, tc: tile.TileContext):
    xT = nc.dram_tensor("xT", [ED, SL], BF16, kind="ExternalInput").ap()
    # wqkm: m-unit-major layout: rows [m*128:(m+1)*128] hold the weights for
    # output-feature block m, free dim = 8 k-blocks of 128 input features.
    wqkm_d = nc.dram_tensor("wqkm", [ED, ED], BF16, kind="ExternalInput").ap()
    bqk_d = nc.dram_tensor("bqk", [PT, NKT], F32, kind="ExternalInput").ap()
    wvb_d = nc.dram_tensor("wvb", [ED, CH], BF16, kind="ExternalInput").ap()
    wp_d = nc.dram_tensor("wproj", [NHC * DH, ED], BF16, kind="ExternalInput").ap()
    out_d = nc.dram_tensor("out", [SL, ED], F32, kind="ExternalOutput").ap()

    res = ctx.enter_context(tc.tile_pool(name="res", bufs=1))
    xt_pool = ctx.enter_context(tc.tile_pool(name="xt", bufs=2))
    q_pool = ctx.enter_context(tc.tile_pool(name="q", bufs=8))
    e_pool = ctx.enter_context(tc.tile_pool(name="e", bufs=8))
    r_pool = ctx.enter_context(tc.tile_pool(name="r", bufs=4))
    b_pool = ctx.enter_context(tc.tile_pool(name="b", bufs=4))
    o_pool = ctx.enter_context(tc.tile_pool(name="o", bufs=4))
    rd_pool = ctx.enter_context(tc.tile_pool(name="rd", bufs=2, space="DRAM"))
    # PSUM: tag "s" = 2 bufs x [128,1024] (2 banks each) shared by attention
    # scores, QKV accumulators and projection accumulators; tag "y" = 4 bufs
    # x 1 bank for the PV accumulators.  4 + 4 = all 8 banks.
    ps = ctx.enter_context(tc.tile_pool(name="ps", bufs=2, space="PSUM"))

    # ---- resident tiles / DMA schedule --------------------------------
    # Order matters: the first QKV unit needs wqkm[0] + the x k-tiles in
    # order, so those go first; the rest of the weights follow.
    bqk_t = res.tile([PT, NKT], F32, tag="bqk")
    nc.sync.dma_start(out=bqk_t, in_=bqk_d)

    wqkm = []
    t = res.tile([PT, ED], BF16, tag="wqkm0", name="wqkm0")
    nc.sync.dma_start(out=t, in_=wqkm_d[0:PT, :])
    wqkm.append(t)

    xts_by_ci = {}
    qtiles_by_ci = {}

    def load_xt(ci):
        c0 = ci * CH
        xts = []
        for k in range(NKT):
            t = xt_pool.tile([PT, CH], BF16, tag=f"xt{k}", name=f"xt{k}_{ci}")
            nc.sync.dma_start(out=t, in_=xT[k * PT:(k + 1) * PT, c0:c0 + CH])
            xts.append(t)
        xts_by_ci[ci] = xts
        qtiles_by_ci[ci] = [None] * 4

    load_xt(0)

    for m in range(1, NKT):
        t = res.tile([PT, ED], BF16, tag=f"wqkm{m}", name=f"wqkm{m}")
        nc.sync.dma_start(out=t, in_=wqkm_d[m * PT:(m + 1) * PT, :])
        wqkm.append(t)

    wv = []
    for k in range(NKT):
        t = res.tile([PT, CH], BF16, tag=f"wv{k}", name=f"wv{k}")
        nc.sync.dma_start(out=t, in_=wvb_d[k * PT:(k + 1) * PT, :])
        wv.append(t)

    # v in [seq, head*65] layout: per head 64 v-dims + a ones column (for the
    # softmax denominator row of the PV matmul).
    vv = []
    for st in range(NST):
        t = res.tile([PT, NHC * (DH + 1)], BF16, tag=f"vv{st}")
        nc.vector.memset(
            t.rearrange("p (h c) -> p h c", c=DH + 1)[:, :, DH:DH + 1], 1.0)
        vv.append(t)

    # k^T resident (bf16): 4 pair-tiles [128, 2048]; q per-chunk via pool
    kt = []
    for p in range(4):
        kt.append(res.tile([PT, SL], BF16, tag=f"kt{p}", name=f"kt{p}"))
    # y^T (normalized) resident bf16: pair p rows = head dims of heads 2p,2p+1
    yt = []
    for p in range(4):
        yt.append(res.tile([PT, SL], BF16, tag=f"yt{p}", name=f"yt{p}"))

    wp = []

    def load_wp():
        for p in range(4):
            t = res.tile([PT, ED], BF16, tag=f"wp{p}", name=f"wp{p}")
            nc.sync.dma_start(out=t, in_=wp_d[p * PT:(p + 1) * PT, :])
            wp.append(t)

    # ---- work units ----------------------------------------------------
    def qkv_unit(ci, m):
        # m in 0..7: q/k projection M-tile; m in 8..11: v projection s-tile
        def f():
            c0 = ci * CH
            xts = xts_by_ci[ci]
            if m < NKT:
                pst = ps.tile([PT, CH], F32, tag="s", name=f"qk{ci}_{m}")
                for k in range(NKT):
                    nc.tensor.matmul(
                        pst, lhsT=wqkm[m][:, k * PT:(k + 1) * PT], rhs=xts[k],
                        start=(k == 0), stop=(k == NKT - 1))
                if m < 4:
                    dst = q_pool.tile([PT, CH], BF16, tag="q",
                                      name=f"q{ci}_{m}")
                    qtiles_by_ci[ci][m] = dst
                else:
                    dst = kt[m - 4][:, c0:c0 + CH]
                nc.vector.tensor_scalar_add(out=dst, in0=pst,
                                            scalar1=bqk_t[:, m:m + 1])
            else:
                st = m - NKT
                s_t = ci * 4 + st
                pst = ps.tile([PT, CH], F32, tag="s", name=f"v{ci}_{st}")
                for k in range(NKT):
                    nc.tensor.matmul(
                        pst, lhsT=xts[k][:, st * PT:(st + 1) * PT], rhs=wv[k],
                        start=(k == 0), stop=(k == NKT - 1))
                nc.vector.tensor_copy(
                    out=vv[s_t].rearrange(
                        "p (h c) -> p h c", c=DH + 1)[:, :, 0:DH],
                    in_=pst.rearrange("p (h c) -> p h c", c=DH))
        return f

    def proj_unit(it, ec):
        def f():
            pst = ps.tile([PT, CH], F32, tag="s", name=f"pj{it}_{ec}")
            for p in range(4):
                nc.tensor.matmul(
                    pst, lhsT=yt[p][:, it * PT:(it + 1) * PT],
                    rhs=wp[p][:, ec * CH:(ec + 1) * CH],
                    start=(p == 0), stop=(p == 3))
            o = o_pool.tile([PT, CH], F32, tag="o", name=f"o{it}_{ec}")
            nc.vector.tensor_copy(out=o, in_=pst)
            nc.sync.dma_start(
                out=out_d[it * PT:(it + 1) * PT, ec * CH:(ec + 1) * CH], in_=o)
        return f

    def attn_pair(ci, p, next_filler):
        qt = qtiles_by_ci[ci][p]
        njt = 4 * ci + 4
        ya = ps.tile([DH + 1, CH], F32, tag="y", bufs=4, name=f"ya{ci}_{p}")
        yb = ps.tile([DH + 1, CH], F32, tag="y", bufs=4, name=f"yb{ci}_{p}")
        s2s, es = {}, {}

        def emit_S(jt):
            c_lo = max(jt - 4 * ci, 0) * PT
            s2 = ps.tile([PT, 2 * CH], F32, tag="s", name=f"s{ci}_{p}_{jt}")
            nc.tensor.matmul(
                s2[:, c_lo:CH], lhsT=kt[p][0:DH, jt * PT:(jt + 1) * PT],
                rhs=qt[0:DH, c_lo:CH], start=True, stop=True,
                skip_group_check=True)
            nc.tensor.matmul(
                s2[:, CH + c_lo:2 * CH], lhsT=kt[p][DH:PT, jt * PT:(jt + 1) * PT],
                rhs=qt[DH:PT, c_lo:CH], start=True, stop=True,
                skip_group_check=True)
            s2s[jt] = (s2, c_lo)

        def emit_exp(jt):
            s2, c_lo = s2s.pop(jt)
            t_d = jt - 4 * ci
            e = e_pool.tile([PT, 2 * CH], BF16, tag="e", name=f"e{ci}_{p}_{jt}")
            sv = s2.rearrange("p (h c) -> p h c", h=2)
            ev = e.rearrange("p (h c) -> p h c", h=2)
            nc.scalar.activation(
                out=ev[:, :, c_lo:CH], in_=sv[:, :, c_lo:CH],
                func=mybir.ActivationFunctionType.Exp, scale=0.125)
            if t_d >= 0:
                # triangle sub-tile [128, 2, 128]: keep (local col) >= partition
                nc.gpsimd.affine_select(
                    out=ev[:, :, c_lo:c_lo + PT],
                    in_=ev[:, :, c_lo:c_lo + PT],
                    compare_op=mybir.AluOpType.is_ge, fill=0.0,
                    base=0, pattern=[[0, 2], [1, PT]],
                    channel_multiplier=-1)
            es[jt] = (e, c_lo)

        def emit_PV(jt, first, last):
            e, c_lo = es.pop(jt)
            va = vv[jt][:, (2 * p) * (DH + 1):(2 * p + 1) * (DH + 1)]
            vb = vv[jt][:, (2 * p + 1) * (DH + 1):(2 * p + 2) * (DH + 1)]
            nc.tensor.matmul(ya[:, c_lo:CH], lhsT=va, rhs=e[:, c_lo:CH],
                             start=first, stop=last, skip_group_check=True)
            nc.tensor.matmul(yb[:, c_lo:CH], lhsT=vb,
                             rhs=e[:, CH + c_lo:2 * CH],
                             start=first, stop=last, skip_group_check=True)

        # Process j-tiles diagonal-first, interleaved with full (sub-diagonal)
        # tiles: the diagonal tiles have the longest dependency chain
        # (S -> exp -> affine_select -> PV), so spacing them out keeps their
        # masking latency covered by neighbouring full tiles' PE work.  The
        # first tile is t_d=0 (full width, carries the accumulation start);
        # PSUM accumulation order is irrelevant (it's a sum).
        diag = [4 * ci + t for t in range(4) if 4 * ci + t < njt]
        full = [j for j in range(njt) if j not in diag]
        order = [diag[0]]
        fi = 0
        for dj in diag[1:]:
            take = min(fi + 2, len(full))
            order.extend(full[fi:take])
            fi = take
            order.append(dj)
        order.extend(full[fi:])

        emit_S(order[0])
        if njt > 1:
            emit_S(order[1])
        emit_exp(order[0])
        for idx, jt in enumerate(order):
            if idx + 2 < njt:
                emit_S(order[idx + 2])
            if idx + 1 < njt:
                emit_exp(order[idx + 1])
            next_filler()
            emit_PV(jt, first=(idx == 0), last=(idx == njt - 1))

        # normalize: yt[p] rows = y / denom.  1/d = exp(-ln d) on the Act
        # engine (ln and exp share one activation table, so no table
        # reloads), broadcast across 64 partitions on GpSimd, multiply the
        # PSUM y rows on DVE.
        c0 = ci * CH
        for half, yh in ((0, ya), (1, yb)):
            ld = r_pool.tile([1, CH], F32, tag="ld", name=f"ld{ci}_{p}_{half}")
            nc.scalar.activation(out=ld, in_=yh[DH:DH + 1, :],
                                 func=mybir.ActivationFunctionType.Ln)
            rc = r_pool.tile([1, CH], F32, tag="rc", name=f"rc{ci}_{p}_{half}")
            nc.scalar.activation(out=rc, in_=ld,
                                 func=mybir.ActivationFunctionType.Exp,
                                 scale=-1.0)
            bc = b_pool.tile([DH, CH], F32, tag="bc", name=f"bc{ci}_{p}_{half}")
            nc.gpsimd.partition_broadcast(bc, rc)
            nc.vector.tensor_mul(
                out=yt[p][half * DH:(half + 1) * DH, c0:c0 + CH],
                in0=yh[0:DH, :], in1=bc)

    # ---- main schedule --------------------------------------------------
    for u in range(12):
        qkv_unit(0, u)()

    for ci in range(NCI):
        if ci + 1 < NCI:
            load_xt(ci + 1)
            fillers = [qkv_unit(ci + 1, u) for u in range(12)]
        else:
            # all proj tiles for already-normalized chunks 0..2
            fillers = [proj_unit(it, ec) for it in range(12) for ec in range(2)]
        if ci == 1:
            load_wp()
        total_jts = 4 * (4 * ci + 4)
        rate = len(fillers) / total_jts
        credit = 0.0

        def next_filler():
            nonlocal credit
            credit += rate
            while credit >= 1.0 and fillers:
                credit -= 1.0
                fillers.pop(0)()

        for p in range(4):
            attn_pair(ci, p, next_filler)
        for f in fillers:
            f()

    for it in range(12, NST):
        for ec in range(2):
            proj_unit(it, ec)()


_CACHED = {}


def _get_nc():
    if "nc" not in _CACHED:
        from contextlib import ExitStack

        from concourse import bacc

        nc = bacc.Bacc("TRN2", target_bir_lowering=False, debug=False,
                       num_devices=8)
        with tile.TileContext(nc) as tc, ExitStack() as ctx:
            build_kernel(ctx, nc, tc)
        nc.compile()
        _CACHED["nc"] = nc
    return _CACHED["nc"]


def make_in_maps(x, W_attn, b_attn, W_proj):
    x = np.asarray(x, np.float32)
    W_attn = np.asarray(W_attn, np.float32)
    b_attn = np.asarray(b_attn, np.float32)
    bf16 = ml_dtypes.bfloat16
    in_maps = []
    for c in range(8):
        b, g = c // 2, c % 2
        xT = x[b].T.astype(bf16)
        wqk = np.concatenate(
            [W_attn[:, 512 * g:512 * g + 512],
             W_attn[:, 1024 + 512 * g:1024 + 512 * g + 512]],
            axis=1)
        # m-unit-major relayout: wqkm[m*128+p, k*128+c] = wqk[k*128+p, m*128+c]
        wqkm = np.ascontiguousarray(
            wqk.reshape(NKT, PT, NKT, PT).transpose(2, 1, 0, 3)
            .reshape(ED, ED)).astype(bf16)
        bqk = np.concatenate(
            [b_attn[512 * g:512 * g + 512],
             b_attn[1024 + 512 * g:1024 + 512 * g + 512]]).reshape(NKT, PT).T
        wvb = W_attn[:, 2048 + 512 * g:2048 + 512 * g + 512].astype(bf16)
        wproj = np.asarray(W_proj, np.float32)[512 * g:512 * g + 512, :]
        in_maps.append({
            "xT": np.ascontiguousarray(xT),
            "wqkm": wqkm,
            "bqk": np.ascontiguousarray(bqk),
            "wvb": np.ascontiguousarray(wvb),
            "wproj": np.ascontiguousarray(wproj.astype(bf16)),
        })
    return in_maps


def run(x, W_attn, b_attn, W_proj, b_proj, **spmd_kwargs):
    nc = _get_nc()
    in_maps = make_in_maps(x, W_attn, b_attn, W_proj)
    res = run_bass_kernel_spmd(nc, in_maps, core_ids=list(range(8)),
                               **spmd_kwargs)
    outs = [r["out"] for r in res.results]
    # v-bias never enters the kernel: y uses (v + bv) only additively, and
    # softmax rows sum to 1, so out += bv @ W_proj folds into the host bias.
    b_eff = (np.asarray(b_proj, np.float32)
             + np.asarray(b_attn, np.float32)[2048:]
             @ np.asarray(W_proj, np.float32))
    out = np.stack([outs[2 * b] + outs[2 * b + 1] + b_eff for b in range(4)])
    return out.astype(np.float32), res


def kernel(x, W_attn, b_attn, W_proj, b_proj):
    out, _ = run(x, W_attn, b_attn, W_proj, b_proj)
    return out


# revision 15
# speedup vs baseline: 1.1223x; 1.1223x over previous
"""Causal self-attention (GPT-style) Bass/Tile kernel for 8 Trainium2 NeuronCores.

Reference computation (fp32):
    qkv = x @ W_attn + b_attn ; q,k,v = split(qkv)
    heads: [B=4, H=16, S=2048, D=64]
    att = softmax(causal(q k^T / sqrt(64)))
    y   = att @ v  -> [B, S, 1024]
    out = y @ W_proj + b_proj

Sharding (hardcoded): 8 cores = 4 batches x 2 head-groups (tensor parallel over
heads).  Core c handles batch c//2, heads 8*(c%2) .. 8*(c%2)+7.  Each core
computes a partial projection output [2048, 1024]; the host sums the two
head-group partials per batch and adds b_proj.

Per-core kernel layout notes:
  - All matmuls run through the PE array as out = lhsT.T @ rhs.  The cost of a
    matmul is proportional to the *output free size* (rhs columns streamed), so
    the kernel trims every attention matmul to the causally-live column range.
  - QKV phase computes q^T / k^T ([feature, seq], feature on partitions) and
    v in [seq, feature] layout, so attention needs no on-chip transposes:
      S^T[j, i] = sum_d kT[d, j] qT[d, i]    (two heads -> two halves of one
                                              2-bank PSUM tile [128, 1024])
      E = exp(S^T / 8) in ONE activation per j-tile (strided AP over both
      heads' live columns); causal triangle masked post-exp (fill 0).
      yT[d, i] (+ row 64 = softmax denom) = [v | 1]^T E  (M=65, K=j)
    Softmax needs no max-subtraction: |S/8| <= ~6 for these inputs.
  - The PE stream is software-pipelined: S(jt+2) issues before PV(jt) so the
    exp of jt overlaps PV(jt-1)/S(jt+1)/filler work instead of stalling PE.
  - Independent "filler" units (next chunk's QKV, or output-projection tiles)
    are paced evenly through the attention j-loops; all projection tiles for
    chunks 0-2 run inside chunk 3 (where the exp deficit is largest).
  - Normalization: denom row -> reciprocal_approx_fast (DVE) ->
    partition_broadcast (GpSimd) -> y * r straight out of PSUM (DVE).
  - bf16 everywhere on the PE; W_proj is cast to bf16 on the host.
"""

import ml_dtypes
import numpy as np

import concourse.bass as bass
import concourse.mybir as mybir
import concourse.tile as tile
from concourse.bass_utils import run_bass_kernel_spmd

F32 = mybir.dt.float32
BF16 = mybir.dt.bfloat16

SL = 2048          # sequence length
ED = 1024          # embed dim
NHC = 8            # heads per core
DH = 64            # head dim
PT = 128           # partitions
CH = 512           # free-dim chunk (PSUM bank)
NCI = SL // CH     # 4 i-chunks
NST = SL // PT     # 16 seq tiles
NKT = ED // PT     # 8 contraction tiles for QKV


def build_kernel(ctx, nc: bass.Bass, tc: tile.TileContext):
    xT = nc.dram_tensor("xT", [ED, SL], BF16, kind="ExternalInput").ap()
    # wqkm: m-unit-major layout: rows [m*128:(m+1)*128] hold the weights for
    # output-feature block m, free dim = 8 k-blocks of 128 input features.
    wqkm_d = nc.dram_tensor("wqkm", [ED, ED], BF16, kind="ExternalInput").ap()
    bqk_d = nc.dram_tensor("bqk", [PT, NKT], F32, kind="ExternalInput").ap()
    wvb_d = nc.dram_tensor("wvb", [ED, CH], BF16, kind="ExternalInput").ap()
    wp_d = nc.dram_tensor("wproj", [NHC * DH, ED], BF16, kind="ExternalInput").ap()
    out_d = nc.dram_tensor("out", [SL, ED], F32, kind="ExternalOutput").ap()

    res = ctx.enter_context(tc.tile_pool(name="res", bufs=1))
    xt_pool = ctx.enter_context(tc.tile_pool(name="xt", bufs=2))
    q_pool = ctx.enter_context(tc.tile_pool(name="q", bufs=8))
    e_pool = ctx.enter_context(tc.tile_pool(name="e", bufs=8))
    r_pool = ctx.enter_context(tc.tile_pool(name="r", bufs=4))
    b_pool = ctx.enter_context(tc.tile_pool(name="b", bufs=4))
    o_pool = ctx.enter_context(tc.tile_pool(name="o", bufs=4))
    rd_pool = ctx.enter_context(tc.tile_pool(name="rd", bufs=2, space="DRAM"))
    # PSUM: tag "s" = 2 bufs x [128,1024] (2 banks each) shared by attention
    # scores, QKV accumulators and projection accumulators; tag "y" = 4 bufs
    # x 1 bank for the PV accumulators.  4 + 4 = all 8 banks.
    ps = ctx.enter_context(tc.tile_pool(name="ps", bufs=2, space="PSUM"))

    # ---- resident tiles / DMA schedule --------------------------------
    # Order matters: the first QKV unit needs wqkm[0] + the x k-tiles in
    # order, so those go first; the rest of the weights follow.
    bqk_t = res.tile([PT, NKT], F32, tag="bqk")
    nc.sync.dma_start(out=bqk_t, in_=bqk_d)

    wqkm = []
    t = res.tile([PT, ED], BF16, tag="wqkm0", name="wqkm0")
    nc.sync.dma_start(out=t, in_=wqkm_d[0:PT, :])
    wqkm.append(t)

    xts_by_ci = {}
    qtiles_by_ci = {}

    def load_xt(ci):
        c0 = ci * CH
        xts = []
        for k in range(NKT):
            t = xt_pool.tile([PT, CH], BF16, tag=f"xt{k}", name=f"xt{k}_{ci}")
            nc.sync.dma_start(out=t, in_=xT[k * PT:(k + 1) * PT, c0:c0 + CH])
            xts.append(t)
        xts_by_ci[ci] = xts
        qtiles_by_ci[ci] = [None] * 4

    load_xt(0)

    for m in range(1, NKT):
        t = res.tile([PT, ED], BF16, tag=f"wqkm{m}", name=f"wqkm{m}")
        nc.sync.dma_start(out=t, in_=wqkm_d[m * PT:(m + 1) * PT, :])
        wqkm.append(t)

    wv = []
    for k in range(NKT):
        t = res.tile([PT, CH], BF16, tag=f"wv{k}", name=f"wv{k}")
        nc.sync.dma_start(out=t, in_=wvb_d[k * PT:(k + 1) * PT, :])
        wv.append(t)

    # v in [seq, head*65] layout: per head 64 v-dims + a ones column (for the
    # softmax denominator row of the PV matmul).
    vv = []
    for st in range(NST):
        t = res.tile([PT, NHC * (DH + 1)], BF16, tag=f"vv{st}")
        nc.vector.memset(
            t.rearrange("p (h c) -> p h c", c=DH + 1)[:, :, DH:DH + 1], 1.0)
        vv.append(t)

    # k^T resident (bf16): 4 pair-tiles [128, 2048]; q per-chunk via pool
    kt = []
    for p in range(4):
        kt.append(res.tile([PT, SL], BF16, tag=f"kt{p}", name=f"kt{p}"))
    # y^T (normalized) resident bf16: pair p rows = head dims of heads 2p,2p+1
    yt = []
    for p in range(4):
        yt.append(res.tile([PT, SL], BF16, tag=f"yt{p}", name=f"yt{p}"))

    wp = []

    def load_wp():
        for p in range(4):
            t = res.tile([PT, ED], BF16, tag=f"wp{p}", name=f"wp{p}")
            nc.sync.dma_start(out=t, in_=wp_d[p * PT:(p + 1) * PT, :])
            wp.append(t)

    # ---- work units ----------------------------------------------------
    def qkv_unit(ci, m):
        # m in 0..7: q/k projection M-tile; m in 8..11: v projection s-tile
        def f():
            c0 = ci * CH
            xts = xts_by_ci[ci]
            if m < NKT:
                pst = ps.tile([PT, CH], F32, tag="s", name=f"qk{ci}_{m}")
                for k in range(NKT):
                    nc.tensor.matmul(
                        pst, lhsT=wqkm[m][:, k * PT:(k + 1) * PT], rhs=xts[k],
                        start=(k == 0), stop=(k == NKT - 1))
                if m < 4:
                    dst = q_pool.tile([PT, CH], BF16, tag="q",
                                      name=f"q{ci}_{m}")
                    qtiles_by_ci[ci][m] = dst
                else:
                    dst = kt[m - 4][:, c0:c0 + CH]
                nc.vector.tensor_scalar_add(out=dst, in0=pst,
                                            scalar1=bqk_t[:, m:m + 1])
            else:
                st = m - NKT
                s_t = ci * 4 + st
                pst = ps.tile([PT, CH], F32, tag="s", name=f"v{ci}_{st}")
                for k in range(NKT):
                    nc.tensor.matmul(
                        pst, lhsT=xts[k][:, st * PT:(st + 1) * PT], rhs=wv[k],
                        start=(k == 0), stop=(k == NKT - 1))
                nc.vector.tensor_copy(
                    out=vv[s_t].rearrange(
                        "p (h c) -> p h c", c=DH + 1)[:, :, 0:DH],
                    in_=pst.rearrange("p (h c) -> p h c", c=DH))
        return f

    def proj_unit(it, ec):
        def f():
            pst = ps.tile([PT, CH], F32, tag="s", name=f"pj{it}_{ec}")
            for p in range(4):
                nc.tensor.matmul(
                    pst, lhsT=yt[p][:, it * PT:(it + 1) * PT],
                    rhs=wp[p][:, ec * CH:(ec + 1) * CH],
                    start=(p == 0), stop=(p == 3))
            o = o_pool.tile([PT, CH], F32, tag="o", name=f"o{it}_{ec}")
            nc.vector.tensor_copy(out=o, in_=pst)
            nc.sync.dma_start(
                out=out_d[it * PT:(it + 1) * PT, ec * CH:(ec + 1) * CH], in_=o)
        return f

    def attn_pair(ci, p, next_filler):
        qt = qtiles_by_ci[ci][p]
        njt = 4 * ci + 4
        ya = ps.tile([DH + 1, CH], F32, tag="y", bufs=4, name=f"ya{ci}_{p}")
        yb = ps.tile([DH + 1, CH], F32, tag="y", bufs=4, name=f"yb{ci}_{p}")
        s2s, es = {}, {}

        def emit_S(jt):
            c_lo = max(jt - 4 * ci, 0) * PT
            s2 = ps.tile([PT, 2 * CH], F32, tag="s", name=f"s{ci}_{p}_{jt}")
            nc.tensor.matmul(
                s2[:, c_lo:CH], lhsT=kt[p][0:DH, jt * PT:(jt + 1) * PT],
                rhs=qt[0:DH, c_lo:CH], start=True, stop=True,
                skip_group_check=True)
            nc.tensor.matmul(
                s2[:, CH + c_lo:2 * CH], lhsT=kt[p][DH:PT, jt * PT:(jt + 1) * PT],
                rhs=qt[DH:PT, c_lo:CH], start=True, stop=True,
                skip_group_check=True)
            s2s[jt] = (s2, c_lo)

        def emit_exp(jt):
            s2, c_lo = s2s.pop(jt)
            t_d = jt - 4 * ci
            e = e_pool.tile([PT, 2 * CH], BF16, tag="e", name=f"e{ci}_{p}_{jt}")
            sv = s2.rearrange("p (h c) -> p h c", h=2)
            ev = e.rearrange("p (h c) -> p h c", h=2)
            nc.scalar.activation(
                out=ev[:, :, c_lo:CH], in_=sv[:, :, c_lo:CH],
                func=mybir.ActivationFunctionType.Exp, scale=0.125)
            if t_d >= 0:
                # triangle sub-tile [128, 2, 128]: keep (local col) >= partition
                nc.gpsimd.affine_select(
                    out=ev[:, :, c_lo:c_lo + PT],
                    in_=ev[:, :, c_lo:c_lo + PT],
                    compare_op=mybir.AluOpType.is_ge, fill=0.0,
                    base=0, pattern=[[0, 2], [1, PT]],
                    channel_multiplier=-1)
            es[jt] = (e, c_lo)

        def emit_PV(jt, first, last):
            e, c_lo = es.pop(jt)
            va = vv[jt][:, (2 * p) * (DH + 1):(2 * p + 1) * (DH + 1)]
            vb = vv[jt][:, (2 * p + 1) * (DH + 1):(2 * p + 2) * (DH + 1)]
            nc.tensor.matmul(ya[:, c_lo:CH], lhsT=va, rhs=e[:, c_lo:CH],
                             start=first, stop=last, skip_group_check=True)
            nc.tensor.matmul(yb[:, c_lo:CH], lhsT=vb,
                             rhs=e[:, CH + c_lo:2 * CH],
                             start=first, stop=last, skip_group_check=True)

        # Process j-tiles diagonal-first, interleaved with full (sub-diagonal)
        # tiles: the diagonal tiles have the longest dependency chain
        # (S -> exp -> affine_select -> PV), so spacing them out keeps their
        # masking latency covered by neighbouring full tiles' PE work.  The
        # first tile is t_d=0 (full width, carries the accumulation start);
        # PSUM accumulation order is irrelevant (it's a sum).
        diag = [4 * ci + t for t in range(4) if 4 * ci + t < njt]
        full = [j for j in range(njt) if j not in diag]
        order = [diag[0]]
        fi = 0
        for dj in diag[1:]:
            take = min(fi + 2, len(full))
            order.extend(full[fi:take])
            fi = take
            order.append(dj)
        order.extend(full[fi:])

        emit_S(order[0])
        if njt > 1:
            emit_S(order[1])
        emit_exp(order[0])
        for idx, jt in enumerate(order):
            if idx + 2 < njt:
                emit_S(order[idx + 2])
            if idx + 1 < njt:
                emit_exp(order[idx + 1])
            next_filler()
            emit_PV(jt, first=(idx == 0), last=(idx == njt - 1))

        # normalize: yt[p] rows = y / denom.  Gather the two denom rows into
        # one SBUF tile (DVE can't write at partition offset 1, so bounce
        # each through a partition-0 staging row + SBUF->SBUF DMA), one DVE
        # reciprocal, then GpSimd partition_broadcast + multiply from PSUM.
        c0 = ci * CH
        coll = r_pool.tile([2, CH], F32, tag="coll", name=f"coll{ci}_{p}")
        for half, yh in ((0, ya), (1, yb)):
            ysb = r_pool.tile([1, CH], F32, tag="ysb", name=f"ysb{ci}_{p}_{half}")
            nc.vector.tensor_copy(out=ysb, in_=yh[DH:DH + 1, :])
            nc.sync.dma_start(out=coll[half:half + 1, :], in_=ysb)
        collr = r_pool.tile([2, CH], F32, tag="collr", name=f"collr{ci}_{p}")
        nc.vector.reciprocal(out=collr, in_=coll)
        rb = r_pool.tile([1, CH], F32, tag="rb", name=f"rb{ci}_{p}")
        nc.sync.dma_start(out=rb, in_=collr[1:2, :])
        for half, yh, rsrc in ((0, ya, collr[0:1, :]), (1, yb, rb)):
            bc = b_pool.tile([DH, CH], F32, tag="bc", name=f"bc{ci}_{p}_{half}")
            nc.gpsimd.partition_broadcast(bc, rsrc)
            nc.vector.tensor_mul(
                out=yt[p][half * DH:(half + 1) * DH, c0:c0 + CH],
                in0=yh[0:DH, :], in1=bc)

    # ---- main schedule --------------------------------------------------
    for u in range(12):
        qkv_unit(0, u)()

    for ci in range(NCI):
        if ci + 1 < NCI:
            load_xt(ci + 1)
            fillers = [qkv_unit(ci + 1, u) for u in range(12)]
        else:
            # all proj tiles for already-normalized chunks 0..2
            fillers = [proj_unit(it, ec) for it in range(12) for ec in range(2)]
        if ci == 1:
            load_wp()
        total_jts = 4 * (4 * ci + 4)
        rate = len(fillers) / total_jts
        credit = 0.0

        def next_filler():
            nonlocal credit
            credit += rate
            while credit >= 1.0 and fillers:
                credit -= 1.0
                fillers.pop(0)()

        for p in range(4):
            attn_pair(ci, p, next_filler)
        for f in fillers:
            f()

    for it in range(12, NST):
        for ec in range(2):
            proj_unit(it, ec)()


_CACHED = {}


def _get_nc():
    if "nc" not in _CACHED:
        from contextlib import ExitStack

        from concourse import bacc

        nc = bacc.Bacc("TRN2", target_bir_lowering=False, debug=False,
                       num_devices=8)
        with tile.TileContext(nc) as tc, ExitStack() as ctx:
            build_kernel(ctx, nc, tc)
        nc.compile()
        _CACHED["nc"] = nc
    return _CACHED["nc"]


def make_in_maps(x, W_attn, b_attn, W_proj):
    x = np.asarray(x, np.float32)
    W_attn = np.asarray(W_attn, np.float32)
    b_attn = np.asarray(b_attn, np.float32)
    bf16 = ml_dtypes.bfloat16
    in_maps = []
    for c in range(8):
        b, g = c // 2, c % 2
        xT = x[b].T.astype(bf16)
        wqk = np.concatenate(
            [W_attn[:, 512 * g:512 * g + 512],
             W_attn[:, 1024 + 512 * g:1024 + 512 * g + 512]],
            axis=1)
        # m-unit-major relayout: wqkm[m*128+p, k*128+c] = wqk[k*128+p, m*128+c]
        wqkm = np.ascontiguousarray(
            wqk.reshape(NKT, PT, NKT, PT).transpose(2, 1, 0, 3)
            .reshape(ED, ED)).astype(bf16)
        bqk = np.concatenate(
            [b_attn[512 * g:512 * g + 512],
             b_attn[1024 + 512 * g:1024 + 512 * g + 512]]).reshape(NKT, PT).T
        wvb = W_attn[:, 2048 + 512 * g:2048 + 512 * g + 512].astype(bf16)
        wproj = np.asarray(W_proj, np.float32)[512 * g:512 * g + 512, :]
        in_maps.append({
            "xT": np.ascontiguousarray(xT),
            "wqkm": wqkm,
            "bqk": np.ascontiguousarray(bqk),
            "wvb": np.ascontiguousarray(wvb),
            "wproj": np.ascontiguousarray(wproj.astype(bf16)),
        })
    return in_maps


def run(x, W_attn, b_attn, W_proj, b_proj, **spmd_kwargs):
    nc = _get_nc()
    in_maps = make_in_maps(x, W_attn, b_attn, W_proj)
    res = run_bass_kernel_spmd(nc, in_maps, core_ids=list(range(8)),
                               **spmd_kwargs)
    outs = [r["out"] for r in res.results]
    # v-bias never enters the kernel: y uses (v + bv) only additively, and
    # softmax rows sum to 1, so out += bv @ W_proj folds into the host bias.
    b_eff = (np.asarray(b_proj, np.float32)
             + np.asarray(b_attn, np.float32)[2048:]
             @ np.asarray(W_proj, np.float32))
    out = np.stack([outs[2 * b] + outs[2 * b + 1] + b_eff for b in range(4)])
    return out.astype(np.float32), res


def kernel(x, W_attn, b_attn, W_proj, b_proj):
    out, _ = run(x, W_attn, b_attn, W_proj, b_proj)
    return out


# revision 24
# speedup vs baseline: 1.2496x; 1.1135x over previous
"""Causal self-attention (GPT-style) Bass/Tile kernel for 8 Trainium2 NeuronCores.

Reference computation (fp32):
    qkv = x @ W_attn + b_attn ; q,k,v = split(qkv)
    heads: [B=4, H=16, S=2048, D=64]
    att = softmax(causal(q k^T / sqrt(64)))
    y   = att @ v  -> [B, S, 1024]
    out = y @ W_proj + b_proj

Sharding (hardcoded): 8 cores = 4 batches x 2 head-groups (tensor parallel over
heads).  Core c handles batch c//2, heads 8*(c%2) .. 8*(c%2)+7.  Each core
computes a partial projection output [2048, 1024]; the host sums the two
head-group partials per batch and adds b_proj.

Per-core kernel layout notes:
  - All matmuls run through the PE array as out = lhsT.T @ rhs.  The cost of a
    matmul is proportional to the *output free size* (rhs columns streamed), so
    the kernel trims every attention matmul to the causally-live column range.
  - QKV phase computes q^T / k^T ([feature, seq], feature on partitions) and
    v in [seq, feature] layout, so attention needs no on-chip transposes:
      S^T[j, i] = sum_d kT[d, j] qT[d, i]    (two heads -> two halves of one
                                              2-bank PSUM tile [128, 1024])
      E = exp(S^T / 8) in ONE activation per j-tile (strided AP over both
      heads' live columns); causal triangle masked post-exp (fill 0).
      yT[d, i] (+ row 64 = softmax denom) = [v | 1]^T E  (M=65, K=j)
    Softmax needs no max-subtraction: |S/8| <= ~6 for these inputs.
  - The PE stream is software-pipelined: S(jt+2) issues before PV(jt) so the
    exp of jt overlaps PV(jt-1)/S(jt+1)/filler work instead of stalling PE.
  - Independent "filler" units (next chunk's QKV, or output-projection tiles)
    are paced evenly through the attention j-loops; all projection tiles for
    chunks 0-2 run inside chunk 3 (where the exp deficit is largest).
  - Normalization: denom row -> reciprocal_approx_fast (DVE) ->
    partition_broadcast (GpSimd) -> y * r straight out of PSUM (DVE).
  - bf16 everywhere on the PE; W_proj is cast to bf16 on the host.
"""

import ml_dtypes
import numpy as np

import concourse.bass as bass
import concourse.mybir as mybir
import concourse.tile as tile
from concourse.bass_utils import run_bass_kernel_spmd

F32 = mybir.dt.float32
BF16 = mybir.dt.bfloat16

SL = 2048          # sequence length
ED = 1024          # embed dim
NHC = 8            # heads per core
DH = 64            # head dim
PT = 128           # partitions
CH = 512           # free-dim chunk (PSUM bank)
NCI = SL // CH     # 4 i-chunks
NST = SL // PT     # 16 seq tiles
NKT = ED // PT     # 8 contraction tiles for QKV


def build_kernel(ctx, nc: bass.Bass, tc: tile.TileContext):
    xT = nc.dram_tensor("xT", [ED, SL], BF16, kind="ExternalInput").ap()
    # wqkm: m-unit-major layout: rows [m*128:(m+1)*128] hold the weights for
    # output-feature block m, free dim = 8 k-blocks of 128 input features.
    wqkm_d = nc.dram_tensor("wqkm", [ED, ED], BF16, kind="ExternalInput").ap()
    bqk_d = nc.dram_tensor("bqk", [PT, NKT], F32, kind="ExternalInput").ap()
    wvb_d = nc.dram_tensor("wvb", [ED, CH], BF16, kind="ExternalInput").ap()
    wp_d = nc.dram_tensor("wproj", [NHC * DH, ED], BF16, kind="ExternalInput").ap()
    out_d = nc.dram_tensor("out", [SL, ED], F32, kind="ExternalOutput").ap()

    res = ctx.enter_context(tc.tile_pool(name="res", bufs=1))
    xt_pool = ctx.enter_context(tc.tile_pool(name="xt", bufs=2))
    q_pool = ctx.enter_context(tc.tile_pool(name="q", bufs=8))
    e_pool = ctx.enter_context(tc.tile_pool(name="e", bufs=8))
    r_pool = ctx.enter_context(tc.tile_pool(name="r", bufs=4))
    b_pool = ctx.enter_context(tc.tile_pool(name="b", bufs=4))
    o_pool = ctx.enter_context(tc.tile_pool(name="o", bufs=4))
    rd_pool = ctx.enter_context(tc.tile_pool(name="rd", bufs=2, space="DRAM"))
    # PSUM: tag "s" = 3 bufs x [128,1024] (2 banks each) shared by attention
    # scores, QKV accumulators and projection accumulators (depth-3 rotation
    # so S(jt+2) never waits on exp(jt)); tag "y" = 2 bufs x 1 bank for the
    # PV accumulators.  6 + 2 = all 8 banks.
    ps = ctx.enter_context(tc.tile_pool(name="ps", bufs=3, space="PSUM"))

    # ---- resident tiles / DMA schedule --------------------------------
    # Order matters: the first QKV unit needs wqkm[0] + the x k-tiles in
    # order, so those go first; the rest of the weights follow.
    bqk_t = res.tile([PT, NKT], F32, tag="bqk")
    nc.sync.dma_start(out=bqk_t, in_=bqk_d)

    wqkm = []
    t = res.tile([PT, ED], BF16, tag="wqkm0", name="wqkm0")
    nc.sync.dma_start(out=t, in_=wqkm_d[0:PT, :])
    wqkm.append(t)

    xts_by_ci = {}
    qtiles_by_ci = {}

    def load_xt(ci):
        c0 = ci * CH
        xts = []
        for k in range(NKT):
            t = xt_pool.tile([PT, CH], BF16, tag=f"xt{k}", name=f"xt{k}_{ci}")
            nc.sync.dma_start(out=t, in_=xT[k * PT:(k + 1) * PT, c0:c0 + CH])
            xts.append(t)
        xts_by_ci[ci] = xts
        qtiles_by_ci[ci] = [None] * 4

    load_xt(0)

    for m in range(1, NKT):
        t = res.tile([PT, ED], BF16, tag=f"wqkm{m}", name=f"wqkm{m}")
        nc.sync.dma_start(out=t, in_=wqkm_d[m * PT:(m + 1) * PT, :])
        wqkm.append(t)

    wv = []
    for k in range(NKT):
        t = res.tile([PT, CH], BF16, tag=f"wv{k}", name=f"wv{k}")
        nc.sync.dma_start(out=t, in_=wvb_d[k * PT:(k + 1) * PT, :])
        wv.append(t)

    # v in [seq, head*65] layout: per head 64 v-dims + a ones column (for the
    # softmax denominator row of the PV matmul).
    vv = []
    for st in range(NST):
        t = res.tile([PT, NHC * (DH + 1)], BF16, tag=f"vv{st}")
        nc.vector.memset(
            t.rearrange("p (h c) -> p h c", c=DH + 1)[:, :, DH:DH + 1], 1.0)
        vv.append(t)

    # k^T resident (bf16): 4 pair-tiles [128, 2048]; q per-chunk via pool
    kt = []
    for p in range(4):
        kt.append(res.tile([PT, SL], BF16, tag=f"kt{p}", name=f"kt{p}"))
    # y^T (normalized) resident bf16: pair p rows = head dims of heads 2p,2p+1
    yt = []
    for p in range(4):
        yt.append(res.tile([PT, SL], BF16, tag=f"yt{p}", name=f"yt{p}"))

    wp = []

    def load_wp():
        for p in range(4):
            t = res.tile([PT, ED], BF16, tag=f"wp{p}", name=f"wp{p}")
            nc.sync.dma_start(out=t, in_=wp_d[p * PT:(p + 1) * PT, :])
            wp.append(t)

    # ---- work units ----------------------------------------------------
    def qkv_unit(ci, m):
        # m in 0..7: q/k projection M-tile; m in 8..11: v projection s-tile
        def f():
            c0 = ci * CH
            xts = xts_by_ci[ci]
            if m < NKT:
                pst = ps.tile([PT, CH], F32, tag="s", name=f"qk{ci}_{m}")
                for k in range(NKT):
                    nc.tensor.matmul(
                        pst, lhsT=wqkm[m][:, k * PT:(k + 1) * PT], rhs=xts[k],
                        start=(k == 0), stop=(k == NKT - 1))
                if m < 4:
                    dst = q_pool.tile([PT, CH], BF16, tag="q",
                                      name=f"q{ci}_{m}")
                    qtiles_by_ci[ci][m] = dst
                else:
                    dst = kt[m - 4][:, c0:c0 + CH]
                nc.vector.tensor_scalar_add(out=dst, in0=pst,
                                            scalar1=bqk_t[:, m:m + 1])
            else:
                st = m - NKT
                s_t = ci * 4 + st
                pst = ps.tile([PT, CH], F32, tag="s", name=f"v{ci}_{st}")
                for k in range(NKT):
                    nc.tensor.matmul(
                        pst, lhsT=xts[k][:, st * PT:(st + 1) * PT], rhs=wv[k],
                        start=(k == 0), stop=(k == NKT - 1))
                nc.vector.tensor_copy(
                    out=vv[s_t].rearrange(
                        "p (h c) -> p h c", c=DH + 1)[:, :, 0:DH],
                    in_=pst.rearrange("p (h c) -> p h c", c=DH))
        return f

    def proj_unit(it, ec):
        def f():
            pst = ps.tile([PT, CH], F32, tag="s", name=f"pj{it}_{ec}")
            for p in range(4):
                nc.tensor.matmul(
                    pst, lhsT=yt[p][:, it * PT:(it + 1) * PT],
                    rhs=wp[p][:, ec * CH:(ec + 1) * CH],
                    start=(p == 0), stop=(p == 3))
            o = o_pool.tile([PT, CH], F32, tag="o", name=f"o{it}_{ec}")
            nc.vector.tensor_copy(out=o, in_=pst)
            nc.sync.dma_start(
                out=out_d[it * PT:(it + 1) * PT, ec * CH:(ec + 1) * CH], in_=o)
        return f

    def attn_pair(ci, p, next_filler):
        qt = qtiles_by_ci[ci][p]
        njt = 4 * ci + 4
        yab = [None, None]
        s2s, es = {}, {}

        def emit_S(jt):
            c_lo = max(jt - 4 * ci, 0) * PT
            s2 = ps.tile([PT, 2 * CH], F32, tag="s", name=f"s{ci}_{p}_{jt}")
            nc.tensor.matmul(
                s2[:, c_lo:CH], lhsT=kt[p][0:DH, jt * PT:(jt + 1) * PT],
                rhs=qt[0:DH, c_lo:CH], start=True, stop=True,
                skip_group_check=True)
            nc.tensor.matmul(
                s2[:, CH + c_lo:2 * CH], lhsT=kt[p][DH:PT, jt * PT:(jt + 1) * PT],
                rhs=qt[DH:PT, c_lo:CH], start=True, stop=True,
                skip_group_check=True)
            s2s[jt] = (s2, c_lo)

        def emit_exp(jt):
            s2, c_lo = s2s.pop(jt)
            t_d = jt - 4 * ci
            e = e_pool.tile([PT, 2 * CH], BF16, tag="e", name=f"e{ci}_{p}_{jt}")
            sv = s2.rearrange("p (h c) -> p h c", h=2)
            ev = e.rearrange("p (h c) -> p h c", h=2)
            nc.scalar.activation(
                out=ev[:, :, c_lo:CH], in_=sv[:, :, c_lo:CH],
                func=mybir.ActivationFunctionType.Exp, scale=0.125)
            if t_d >= 0:
                # triangle sub-tile [128, 2, 128]: keep (local col) >= partition
                nc.gpsimd.affine_select(
                    out=ev[:, :, c_lo:c_lo + PT],
                    in_=ev[:, :, c_lo:c_lo + PT],
                    compare_op=mybir.AluOpType.is_ge, fill=0.0,
                    base=0, pattern=[[0, 2], [1, PT]],
                    channel_multiplier=-1)
            es[jt] = (e, c_lo)

        def emit_PV(jt, first, last):
            e, c_lo = es.pop(jt)
            if first:
                # allocate the PV accumulators as late as possible: with
                # bufs=2 the slots are still held by the previous pair until
                # its staging copies finish.
                yab[0] = ps.tile([DH + 1, CH], F32, tag="y", bufs=2,
                                 name=f"ya{ci}_{p}")
                yab[1] = ps.tile([DH + 1, CH], F32, tag="y", bufs=2,
                                 name=f"yb{ci}_{p}")
            va = vv[jt][:, (2 * p) * (DH + 1):(2 * p + 1) * (DH + 1)]
            vb = vv[jt][:, (2 * p + 1) * (DH + 1):(2 * p + 2) * (DH + 1)]
            nc.tensor.matmul(yab[0][:, c_lo:CH], lhsT=va, rhs=e[:, c_lo:CH],
                             start=first, stop=last, skip_group_check=True)
            nc.tensor.matmul(yab[1][:, c_lo:CH], lhsT=vb,
                             rhs=e[:, CH + c_lo:2 * CH],
                             start=first, stop=last, skip_group_check=True)

        # Process j-tiles diagonal-first, interleaved with full (sub-diagonal)
        # tiles: the diagonal tiles have the longest dependency chain
        # (S -> exp -> affine_select -> PV), so spacing them out keeps their
        # masking latency covered by neighbouring full tiles' PE work.  The
        # first tile is t_d=0 (full width, carries the accumulation start);
        # PSUM accumulation order is irrelevant (it's a sum).
        diag = [4 * ci + t for t in range(4) if 4 * ci + t < njt]
        full = [j for j in range(njt) if j not in diag]
        order = [diag[0]]
        fi = 0
        for dj in diag[1:]:
            take = min(fi + 2, len(full))
            order.extend(full[fi:take])
            fi = take
            order.append(dj)
        order.extend(full[fi:])

        emit_S(order[0])
        if njt > 1:
            emit_S(order[1])
        emit_exp(order[0])
        for idx, jt in enumerate(order):
            if idx + 2 < njt:
                emit_S(order[idx + 2])
            if idx + 1 < njt:
                emit_exp(order[idx + 1])
            next_filler()
            emit_PV(jt, first=(idx == 0), last=(idx == njt - 1))

        # Stage [denom; y] to SBUF with one copy per half — this frees the
        # PSUM accumulators within ~1.6us so the next pair's PV never waits.
        # Then normalize off the critical path: fast DVE reciprocal of the
        # partition-0 denom row, GpSimd partition_broadcast, multiply.
        c0 = ci * CH
        for half in range(2):
            dn = r_pool.tile([1, CH], F32, tag="dn", name=f"dn{ci}_{p}_{half}")
            nc.vector.tensor_copy(out=dn, in_=yab[half][DH:DH + 1, :])
            ysb = r_pool.tile([DH, CH], F32, tag="ysb",
                              name=f"ysb{ci}_{p}_{half}")
            nc.vector.tensor_copy(out=ysb, in_=yab[half][0:DH, :])
            rc = r_pool.tile([1, CH], F32, tag="rc", name=f"rc{ci}_{p}_{half}")
            nc.vector.reciprocal_approx_fast(out=rc, in_=dn)
            bc = b_pool.tile([DH, CH], F32, tag="bc", name=f"bc{ci}_{p}_{half}")
            nc.gpsimd.partition_broadcast(bc, rc)
            nc.vector.tensor_mul(
                out=yt[p][half * DH:(half + 1) * DH, c0:c0 + CH],
                in0=ysb, in1=bc)

    # ---- main schedule --------------------------------------------------
    for u in range(12):
        qkv_unit(0, u)()

    for ci in range(NCI):
        if ci + 1 < NCI:
            load_xt(ci + 1)
            fillers = [qkv_unit(ci + 1, u) for u in range(12)]
        else:
            # all proj tiles for already-normalized chunks 0..2
            fillers = [proj_unit(it, ec) for it in range(12) for ec in range(2)]
        if ci == 1:
            load_wp()
        total_jts = 4 * (4 * ci + 4)
        rate = len(fillers) / total_jts
        credit = 0.0

        def next_filler():
            nonlocal credit
            credit += rate
            while credit >= 1.0 and fillers:
                credit -= 1.0
                fillers.pop(0)()

        for p in range(4):
            attn_pair(ci, p, next_filler)
        for f in fillers:
            f()

    for it in range(12, NST):
        for ec in range(2):
            proj_unit(it, ec)()


_CACHED = {}


def _get_nc():
    if "nc" not in _CACHED:
        from contextlib import ExitStack

        from concourse import bacc

        nc = bacc.Bacc("TRN2", target_bir_lowering=False, debug=False,
                       num_devices=8)
        with tile.TileContext(nc) as tc, ExitStack() as ctx:
            build_kernel(ctx, nc, tc)
        nc.compile()
        _CACHED["nc"] = nc
    return _CACHED["nc"]


def make_in_maps(x, W_attn, b_attn, W_proj):
    x = np.asarray(x, np.float32)
    W_attn = np.asarray(W_attn, np.float32)
    b_attn = np.asarray(b_attn, np.float32)
    bf16 = ml_dtypes.bfloat16
    in_maps = []
    for c in range(8):
        b, g = c // 2, c % 2
        xT = x[b].T.astype(bf16)
        wqk = np.concatenate(
            [W_attn[:, 512 * g:512 * g + 512],
             W_attn[:, 1024 + 512 * g:1024 + 512 * g + 512]],
            axis=1)
        # m-unit-major relayout: wqkm[m*128+p, k*128+c] = wqk[k*128+p, m*128+c]
        wqkm = np.ascontiguousarray(
            wqk.reshape(NKT, PT, NKT, PT).transpose(2, 1, 0, 3)
            .reshape(ED, ED)).astype(bf16)
        bqk = np.concatenate(
            [b_attn[512 * g:512 * g + 512],
             b_attn[1024 + 512 * g:1024 + 512 * g + 512]]).reshape(NKT, PT).T
        wvb = W_attn[:, 2048 + 512 * g:2048 + 512 * g + 512].astype(bf16)
        wproj = np.asarray(W_proj, np.float32)[512 * g:512 * g + 512, :]
        in_maps.append({
            "xT": np.ascontiguousarray(xT),
            "wqkm": wqkm,
            "bqk": np.ascontiguousarray(bqk),
            "wvb": np.ascontiguousarray(wvb),
            "wproj": np.ascontiguousarray(wproj.astype(bf16)),
        })
    return in_maps


def run(x, W_attn, b_attn, W_proj, b_proj, **spmd_kwargs):
    nc = _get_nc()
    in_maps = make_in_maps(x, W_attn, b_attn, W_proj)
    res = run_bass_kernel_spmd(nc, in_maps, core_ids=list(range(8)),
                               **spmd_kwargs)
    outs = [r["out"] for r in res.results]
    # v-bias never enters the kernel: y uses (v + bv) only additively, and
    # softmax rows sum to 1, so out += bv @ W_proj folds into the host bias.
    b_eff = (np.asarray(b_proj, np.float32)
             + np.asarray(b_attn, np.float32)[2048:]
             @ np.asarray(W_proj, np.float32))
    out = np.stack([outs[2 * b] + outs[2 * b + 1] + b_eff for b in range(4)])
    return out.astype(np.float32), res


def kernel(x, W_attn, b_attn, W_proj, b_proj):
    out, _ = run(x, W_attn, b_attn, W_proj, b_proj)
    return out
